# revision 38
# baseline (speedup 1.0000x reference)
"""Contrastive patch loss (InfoNCE over sampled voxel patches) on 8 TRN2 NeuronCores.

Math
----
Every sampled voxel index lives in [0, 512), so cs is a gather of the 512x512
Gram matrix G_b = t2n^T @ t1n: cs[k,l] = G_b[i_k, i_l].  With E_b = exp(G_b/bw)
and c_p[s] = multiplicity of voxel s in patch p:

    loss = -1/(P*B*K) * sum_{b,p,s} c_p[s] *
           log(0.5*diagE_b[s]*(1/CS_b[s,p] + 1/RS_b[s,p]) + eps)

where RS_b = E_b @ C^T and CS_b = E_b^T @ C^T.

Sharding: 8 cores = 2 batches x 4 s-row blocks (m).  Core (b,m) computes only
the m-th 128-row block of RS/CS.  The needed lhsT operands E[m,:]^T and
(E^T)[m,:]^T are produced DIRECTLY as col-block Grams (exp of t1^T@t2n[:,m]
resp. t2^T@t1n[:,m]); no PE transposes or PSUM->SBUF copies.  One SPMD
program: all m-dependence enters via host-prepared inputs (m-slice features,
m-slice counts); diag(E)[m-block] comes from the elementwise product of the
two pre-scaled m-slices.  Per-core (128,1) partials; host sums and scales.

DMA is descriptor-bound (~60ns per <=1KB descriptor per queue), so the small
inputs (both m-slices + m-counts) are packed into ONE byte tensor with 768B
rows and bitcast on device; features are fp8e4 (half the descriptors of bf16,
2x PE rate via DoubleRow).  Verified ~2e-4 rel err.
"""

import math

import ml_dtypes
import numpy as np

import concourse.bacc as bacc
import concourse.tile as tile
from concourse import hw_specs, mybir
from concourse.bass_utils import run_bass_kernel_spmd

# Pin every ACTIVATE to the one table set that holds ln+exp+copy so the kernel
# pays a single ACT_TABLE_LOAD.
_PIN_SET = "natural_log_exp_and_others"
_orig_get_tables = hw_specs.get_activation_tables


def _pinned_tables(arch):
    tabs = _orig_get_tables(arch)
    return {k: (v if k == _PIN_SET else set()) for k, v in tabs.items()}


bacc.get_activation_tables = _pinned_tables

B, C, S = 2, 256, 512
P, K = 128, 512
BW = 0.05
EPS = 1e-5
N_CORES = 8
F32 = mybir.dt.float32
BF16 = mybir.dt.bfloat16
FP8 = mybir.dt.float8e4
U8 = mybir.dt.uint8
AX = mybir.AxisListType.X
MUL = mybir.AluOpType.mult
ADD = mybir.AluOpType.add
EXP = mybir.ActivationFunctionType.Exp
LN = mybir.ActivationFunctionType.Ln


FLOOR_FULL = 0.008
FLOOR_FULL2 = 0.008
FLOOR_GRAM = 0.010
FLOOR_DIAG = 0.012
FLOOR_LOSS = 0.014


def _build_program():
    nc = bacc.Bacc("TRN2", target_bir_lowering=False, debug=False, num_devices=N_CORES)

    # combo rows: [f2m fp8 256B | f1m fp8 256B | cntm bf16 256B]
    combod = nc.dram_tensor("combo", [128, 768], U8, kind="ExternalInput")
    # features packed (q, i*512+s): channel c = 128*i + q
    feat2 = nc.dram_tensor("feat2", [128, 1024], FP8, kind="ExternalInput")
    feat1 = nc.dram_tensor("feat1", [128, 1024], FP8, kind="ExternalInput")
    # counts^T blocks (q, a*128+p) = counts[p, 128*a+q]
    cntd = nc.dram_tensor("cnt", [128, 512], BF16, kind="ExternalInput")
    partial = nc.dram_tensor("partial", [1, 1], F32, kind="ExternalOutput")

    with tile.TileContext(nc) as tc:
        with (
            tc.tile_pool(name="const", bufs=1) as const,
            tc.tile_pool(name="feat", bufs=1) as featp,
            tc.tile_pool(name="sq", bufs=1) as sqp,
            tc.tile_pool(name="e", bufs=1) as ep,
            tc.tile_pool(name="small", bufs=1) as small,
            tc.tile_pool(name="loss", bufs=1) as lossp,
            tc.tile_pool(name="ps_g", bufs=3, space="PSUM") as ps_g,
            tc.tile_pool(name="ps_rc", bufs=1, space="PSUM") as ps_rc,
            tc.tile_pool(name="ps_sm", bufs=1, space="PSUM") as ps_sm,
        ):
            # ---- inputs (combo first: it gates the critical chain) ----
            combo = featp.tile([128, 768], U8, name="combo", tag="combo")
            f2 = featp.tile([128, 1024], FP8, name="f2", tag="f2")
            f1 = featp.tile([128, 1024], FP8, name="f1", tag="f1")
            cnt = featp.tile([128, 512], BF16, name="cnt", tag="cnt")
            # issue from four different engines so the ~650ns dma_start
            # executions overlap instead of serializing on Sync
            nc.sync.dma_start(out=combo[:, 0:512], in_=combod[:, 0:512])
            nc.scalar.dma_start(out=f2, in_=feat2[:, :])
            nc.sync.dma_start(out=f1, in_=feat1[:, :])
            nc.gpsimd.dma_start(out=cnt, in_=cntd[:, :])
            nc.gpsimd.dma_start(out=combo[:, 512:768], in_=combod[:, 512:768])
            f2m = combo[:, 0:256].bitcast(FP8)
            f1m = combo[:, 256:512].bitcast(FP8)
            cntm_bc = combo[:, 512:768].bitcast(BF16)

            # ---- constants ----
            ones_col = const.tile([128, 1], BF16, name="ones_col", tag="oc")
            nc.vector.memset(ones_col, 1.0)
            ones_row = const.tile([1, 128], BF16, name="ones_row", tag="orow")
            nc.vector.memset(ones_row, 1.0)
            one_1x1 = const.tile([1, 1], F32, name="one11", tag="one11")
            nc.vector.memset(one_1x1, 1.0)
            eps_col = const.tile([128, 1], F32, name="eps_col", tag="eps")
            nc.vector.memset(eps_col, EPS)
            ln_ibw_col = const.tile([128, 1], F32, name="ln_ibw", tag="libw")
            nc.vector.memset(ln_ibw_col, math.log(1.0 / BW))
            ln_half_col = const.tile([128, 1], F32, name="ln_half", tag="lhalf")
            nc.vector.memset(ln_half_col, math.log(0.5))
            ones8 = const.tile([128, 2], FP8, name="ones8", tag="ones8")
            nc.vector.memset(ones8, 1.0)
            ones8r = ones8.rearrange("q (i o) -> q i o", i=2)

            # shared PSUM bank-tiles (sub-sliced; each distinct tile = 1 bank)
            smallrow = ps_sm.tile([1, 512], F32, name="smallrow", tag="smrow")
            smallcol = ps_sm.tile([128, 512], F32, name="smallcol", tag="smcol")

            # ==== critical m-slice norm path (floor 0) ====
            sqm = sqp.tile([128, 512], BF16, name="sqm", tag="sqm")
            sq2m, sq1m = sqm[:, 0:256], sqm[:, 256:512]
            nc.vector.tensor_tensor(out=sq2m, in0=f2m, in1=f2m, op=MUL)
            nc.vector.tensor_tensor(out=sq1m, in0=f1m, in1=f1m, op=MUL)
            ssm_ps = smallrow[0:1, 0:256]
            for i in range(2):
                nc.tensor.matmul(
                    out=ssm_ps[0:1, 0:128], lhsT=ones_col,
                    rhs=sq2m[:, 128 * i : 128 * (i + 1)],
                    start=(i == 0), stop=(i == 1),
                )
            for i in range(2):
                nc.tensor.matmul(
                    out=ssm_ps[0:1, 128:256], lhsT=ones_col,
                    rhs=sq1m[:, 128 * i : 128 * (i + 1)],
                    start=(i == 0), stop=(i == 1),
                )
            lnm = small.tile([1, 256], F32, name="lnm", tag="lnm")
            nc.scalar.activation(out=lnm, in_=ssm_ps, func=LN)
            invm_row = small.tile([1, 256], BF16, name="invm_row", tag="invm")
            nc.scalar.activation(out=invm_row, in_=lnm, func=EXP, scale=-0.5)
            bc_ps = smallcol[:, 0:256]
            nc.tensor.matmul(out=bc_ps, lhsT=ones_row, rhs=invm_row)
            # pre-scaled m-slices (gram rhs): f2ms = t2m * inv2m, f1ms = t1m * inv1m
            f2ms = featp.tile([128, 256], FP8, name="f2ms", tag="f2ms")
            f1ms = featp.tile([128, 256], FP8, name="f1ms", tag="f1ms")
            for i in range(2):
                isl = slice(128 * i, 128 * (i + 1))
                nc.vector.tensor_tensor(
                    out=f1ms[:, isl], in0=f1m[:, isl], in1=bc_ps[:, 128:256], op=MUL
                )
            for i in range(2):
                isl = slice(128 * i, 128 * (i + 1))
                nc.vector.tensor_tensor(
                    out=f2ms[:, isl], in0=f2m[:, isl], in1=bc_ps[:, 0:128], op=MUL
                )
            dprod = sqp.tile([128, 256], BF16, name="dprod", tag="dprod")
            nc.gpsimd.tensor_tensor(out=dprod, in0=f2ms, in1=f1ms, op=MUL)

            # ==== full-range sumsq, DIRECTLY in column form ====
            # ss_col[s' in a-block] = sum_c sq[c, s'] via 1-column DoubleRow
            # matmuls (lhsT = squares, rhs = ones): no (1,512) Lns, no PE
            # row->col transposes.  t2 squares on Vector (gates the et side),
            # t1 squares on GpSimd (slower but parallel; em side comes later).
            sq2 = sqp.tile([128, 1024], FP8, name="sq2", tag="sq2")
            sq1 = sqp.tile([128, 1024], FP8, name="sq1", tag="sq1")
            nc.vector.tensor_tensor(
                out=sq2[:, 0:512], in0=f2[:, 0:512], in1=f2[:, 0:512], op=MUL
            )
            nc.vector.tensor_tensor(
                out=sq2[:, 512:1024], in0=f2[:, 512:1024], in1=f2[:, 512:1024], op=MUL
            )
            nc.gpsimd.tensor_tensor(
                out=sq1[:, 0:512], in0=f1[:, 0:512], in1=f1[:, 0:512], op=MUL
            )
            nc.gpsimd.tensor_tensor(
                out=sq1[:, 512:1024], in0=f1[:, 512:1024], in1=f1[:, 512:1024], op=MUL
            )
            cntm = lossp.tile([128, 128], F32, name="cntm", tag="cntm")
            nc.gpsimd.tensor_copy(out=cntm, in_=cntm_bc)
            lncol = small.tile([128, 8], F32, name="lncol", tag="lncol")
            invbw = small.tile([128, 8], F32, name="invbw", tag="invbw")
            inv2bw, inv1bw = invbw[:, 0:4], invbw[:, 4:8]
            ss_ps = ps_sm.tile([128, 8], F32, name="ss_ps", tag="ssps")
            with tc.tile_wait_until(FLOOR_FULL):
                sq2r = sq2.rearrange("q (i s) -> q i s", i=2)
                for a in range(4):
                    nc.tensor.matmul(
                        out=ss_ps[:, a : a + 1],
                        lhsT=sq2r[:, :, 128 * a : 128 * (a + 1)],
                        rhs=ones8r,
                        perf_mode=mybir.MatmulPerfMode.DoubleRow,
                    )
                nc.scalar.activation(out=lncol[:, 0:4], in_=ss_ps[:, 0:4], func=LN)
                nc.scalar.activation(
                    out=inv2bw, in_=lncol[:, 0:4], func=EXP, scale=-0.5,
                    bias=ln_ibw_col,
                )
            with tc.tile_wait_until(FLOOR_FULL2):
                sq1r = sq1.rearrange("q (i s) -> q i s", i=2)
                for a in range(4):
                    nc.tensor.matmul(
                        out=ss_ps[:, 4 + a : 5 + a],
                        lhsT=sq1r[:, :, 128 * a : 128 * (a + 1)],
                        rhs=ones8r,
                        perf_mode=mybir.MatmulPerfMode.DoubleRow,
                    )
                nc.scalar.activation(out=lncol[:, 4:8], in_=ss_ps[:, 4:8], func=LN)
                nc.scalar.activation(
                    out=inv1bw, in_=lncol[:, 4:8], func=EXP, scale=-0.5,
                    bias=ln_ibw_col,
                )

            # ==== Grams via fp8 DoubleRow (floor 2) ====
            # Per-block row scales are folded in by Vector tensor_scalar ops
            # (PSUM -> SBUF f32), so each side needs ONE big (128,512) exp
            # instead of four small ones (Scalar was the pipeline bottleneck).
            with tc.tile_wait_until(FLOOR_GRAM):
                et = ep.tile([128, 512], BF16, name="et", tag="et")
                em = ep.tile([128, 512], BF16, name="em", tag="em")
                gsc_et = ep.tile([128, 512], F32, name="gsc_et", tag="gsc_et")
                gsc_em = ep.tile([128, 512], F32, name="gsc_em", tag="gsc_em")
                f2r = f2.rearrange("q (i s) -> q i s", i=2)
                f1r = f1.rearrange("q (i s) -> q i s", i=2)
                f2msr = f2ms.rearrange("q (i j) -> q i j", i=2)
                f1msr = f1ms.rearrange("q (i j) -> q i j", i=2)
                for dst, gsc, lhs_r, rhs_r, sc in (
                    (et, gsc_et, f2r, f1msr, inv2bw),
                    (em, gsc_em, f1r, f2msr, inv1bw),
                ):
                    for a in range(4):
                        g_ps = ps_g.tile([128, 128], F32, name="g_ps", tag="g_ps")
                        nc.tensor.matmul(
                            out=g_ps,
                            lhsT=lhs_r[:, :, 128 * a : 128 * (a + 1)],
                            rhs=rhs_r,
                            perf_mode=mybir.MatmulPerfMode.DoubleRow,
                        )
                        nc.vector.tensor_scalar_mul(
                            out=gsc[:, 128 * a : 128 * (a + 1)], in0=g_ps,
                            scalar1=sc[:, a : a + 1],
                        )
                    nc.scalar.activation(
                        out=dst[:, 0:256], in_=gsc[:, 0:256], func=EXP
                    )
                    nc.scalar.activation(
                        out=dst[:, 256:512], in_=gsc[:, 256:512], func=EXP
                    )

            # ==== diag path (floor 3; only needed by the final Ln) ====
            with tc.tile_wait_until(FLOOR_DIAG):
                dps = smallrow[0:1, 256:384]
                for i in range(2):
                    nc.tensor.matmul(
                        out=dps, lhsT=ones_col, rhs=dprod[:, 128 * i : 128 * (i + 1)],
                        start=(i == 0), stop=(i == 1),
                    )
                drow = small.tile([1, 128], F32, name="drow", tag="drow")
                nc.vector.tensor_copy(out=drow, in_=dps)
                dcol_ps = smallcol[:, 264:265]
                nc.tensor.transpose(out=dcol_ps, in_=drow, identity=one_1x1)
                half_dcol = small.tile([128, 1], F32, name="half_dcol", tag="hdc")
                nc.scalar.activation(
                    out=half_dcol, in_=dcol_ps, func=EXP, scale=1.0 / BW,
                    bias=ln_half_col,
                )

            # ==== RS/CS and loss (floor 4) ====
            with tc.tile_wait_until(FLOOR_LOSS):
                cs_ps = ps_rc.tile([128, 128], F32, name="cs_ps", tag="cs_ps")
                rs_ps = ps_rc.tile([128, 128], F32, name="rs_ps", tag="rs_ps")
                for a in range(4):
                    asl = slice(128 * a, 128 * (a + 1))
                    nc.tensor.matmul(
                        out=cs_ps, lhsT=et[:, asl], rhs=cnt[:, asl],
                        start=(a == 0), stop=(a == 3),
                    )
                for a in range(4):
                    asl = slice(128 * a, 128 * (a + 1))
                    nc.tensor.matmul(
                        out=rs_ps, lhsT=em[:, asl], rhs=cnt[:, asl],
                        start=(a == 0), stop=(a == 3),
                    )
                cinv = lossp.tile([128, 128], F32, name="cinv", tag="cinv")
                rinv = lossp.tile([128, 128], F32, name="rinv", tag="rinv")
                nc.vector.reciprocal_approx_fast(out=cinv, in_=cs_ps)
                nc.vector.reciprocal_approx_fast(out=rinv, in_=rs_ps)
                ssum = lossp.tile([128, 128], F32, name="ssum", tag="ssum")
                nc.vector.tensor_tensor(out=ssum, in0=rinv, in1=cinv, op=ADD)
                g = lossp.tile([128, 128], F32, name="g", tag="g")
                nc.scalar.activation(
                    out=g, in_=ssum, func=LN, scale=half_dcol, bias=eps_col
                )
                scr = lossp.tile([128, 128], BF16, name="scr", tag="scr")
                nc.vector.tensor_tensor(out=scr, in0=g, in1=cntm, op=MUL)
                tot_ps = smallrow[0:1, 384:512]
                nc.tensor.matmul(out=tot_ps, lhsT=ones_col, rhs=scr)
                tot = small.tile([1, 1], F32, name="tot", tag="totsb")
                nc.vector.tensor_reduce(out=tot, in_=tot_ps, axis=AX, op=ADD)
                nc.sync.dma_start(out=partial[:, :], in_=tot)

    nc.compile()
    return nc


_NC = None


def _run(t2_feat, t1_feat, idx, trace=False, trace_kwargs=None):
    global _NC
    if _NC is None:
        _NC = _build_program()

    t2 = np.asarray(t2_feat, np.float32).reshape(B, C, S)
    t1 = np.asarray(t1_feat, np.float32).reshape(B, C, S)
    idx = np.asarray(idx)

    counts = np.zeros((P, S), np.float32)
    np.add.at(counts, (np.arange(P)[:, None], idx), 1.0)
    cnt_dev = np.ascontiguousarray(
        counts.T.reshape(4, 128, 128).transpose(1, 0, 2).reshape(128, 512)
    ).astype(ml_dtypes.bfloat16)

    packed = {}
    for nm, t in (("2", t2), ("1", t1)):
        packed[nm] = [
            np.ascontiguousarray(
                t[b].reshape(2, 128, 512).transpose(1, 0, 2).reshape(128, 1024)
            ).astype(ml_dtypes.float8_e4m3)
            for b in range(B)
        ]

    in_maps = []
    for core in range(N_CORES):
        b, m = divmod(core, 4)
        p2, p1 = packed["2"][b], packed["1"][b]
        msl = [slice(512 * i + 128 * m, 512 * i + 128 * (m + 1)) for i in range(2)]
        f2m = np.concatenate([p2[:, s] for s in msl], axis=1)
        f1m = np.concatenate([p1[:, s] for s in msl], axis=1)
        cntm = np.ascontiguousarray(
            counts[:, 128 * m : 128 * (m + 1)].T
        ).astype(ml_dtypes.bfloat16)
        combo = np.concatenate(
            [
                f2m.view(np.uint8),
                f1m.view(np.uint8),
                cntm.view(np.uint8).reshape(128, 256),
            ],
            axis=1,
        )
        in_maps.append(
            {
                "combo": np.ascontiguousarray(combo),
                "feat2": p2,
                "feat1": p1,
                "cnt": cnt_dev,
            }
        )

    kwargs = {}
    if trace:
        kwargs = dict(trace=True, trace_kwargs=trace_kwargs or {})
    res = run_bass_kernel_spmd(_NC, in_maps, core_ids=list(range(N_CORES)), **kwargs)
    total = sum(float(r["partial"][0, 0]) for r in res.results)
    loss = -total / (P * B * K)
    return np.array(loss, dtype=np.float32), res


def kernel(t2_feat, t1_feat, idx):
    out, _ = _run(t2_feat, t1_feat, idx)
    return out


# revision 41
# speedup vs baseline: 1.0459x; 1.0459x over previous
"""Contrastive patch loss (InfoNCE over sampled voxel patches) on 8 TRN2 NeuronCores.

Math
----
Every sampled voxel index lives in [0, 512), so cs is a gather of the 512x512
Gram matrix G_b = t2n^T @ t1n: cs[k,l] = G_b[i_k, i_l].  With E_b = exp(G_b/bw)
and c_p[s] = multiplicity of voxel s in patch p:

    loss = -1/(P*B*K) * sum_{b,p,s} c_p[s] *
           log(0.5*diagE_b[s]*(1/CS_b[s,p] + 1/RS_b[s,p]) + eps)

where RS_b = E_b @ C^T and CS_b = E_b^T @ C^T.

Sharding: 8 cores = 2 batches x 4 s-row blocks (m).  Core (b,m) computes only
the m-th 128-row block of RS/CS.  The needed lhsT operands E[m,:]^T and
(E^T)[m,:]^T are produced DIRECTLY as col-block Grams (exp of t1^T@t2n[:,m]
resp. t2^T@t1n[:,m]); no PE transposes or PSUM->SBUF copies.  One SPMD
program: all m-dependence enters via host-prepared inputs (m-slice features,
m-slice counts); diag(E)[m-block] comes from the elementwise product of the
two pre-scaled m-slices.  Per-core (128,1) partials; host sums and scales.

DMA is descriptor-bound (~60ns per <=1KB descriptor per queue), so the small
inputs (both m-slices + m-counts) are packed into ONE byte tensor with 768B
rows and bitcast on device; features are fp8e4 (half the descriptors of bf16,
2x PE rate via DoubleRow).  Verified ~2e-4 rel err.
"""

import math

import ml_dtypes
import numpy as np

import concourse.bacc as bacc
import concourse.tile as tile
from concourse import hw_specs, mybir
from concourse.bass_utils import run_bass_kernel_spmd

# Pin every ACTIVATE to the one table set that holds ln+exp+copy so the kernel
# pays a single ACT_TABLE_LOAD.
_PIN_SET = "natural_log_exp_and_others"
_orig_get_tables = hw_specs.get_activation_tables


def _pinned_tables(arch):
    tabs = _orig_get_tables(arch)
    return {k: (v if k == _PIN_SET else set()) for k, v in tabs.items()}


bacc.get_activation_tables = _pinned_tables

B, C, S = 2, 256, 512
P, K = 128, 512
BW = 0.05
EPS = 1e-5
N_CORES = 8
F32 = mybir.dt.float32
BF16 = mybir.dt.bfloat16
FP8 = mybir.dt.float8e4
U8 = mybir.dt.uint8
AX = mybir.AxisListType.X
MUL = mybir.AluOpType.mult
ADD = mybir.AluOpType.add
EXP = mybir.ActivationFunctionType.Exp
LN = mybir.ActivationFunctionType.Ln


FLOOR_FULL = 0.008
FLOOR_FULL2 = 0.008
FLOOR_GRAM = 0.010
FLOOR_DIAG = 0.012
FLOOR_LOSS = 0.014


def _build_program():
    nc = bacc.Bacc("TRN2", target_bir_lowering=False, debug=False, num_devices=N_CORES)

    # combo rows: [f2m fp8 256B | f1m fp8 256B | cntm bf16 256B]
    combod = nc.dram_tensor("combo", [128, 768], U8, kind="ExternalInput")
    # features packed (q, i*512+s): channel c = 128*i + q
    feat2 = nc.dram_tensor("feat2", [128, 1024], FP8, kind="ExternalInput")
    feat1 = nc.dram_tensor("feat1", [128, 1024], FP8, kind="ExternalInput")
    # counts^T blocks (q, a*128+p) = counts[p, 128*a+q]
    cntd = nc.dram_tensor("cnt", [128, 512], BF16, kind="ExternalInput")
    partial = nc.dram_tensor("partial", [1, 1], F32, kind="ExternalOutput")

    with tile.TileContext(nc) as tc:
        with (
            tc.tile_pool(name="const", bufs=1) as const,
            tc.tile_pool(name="feat", bufs=1) as featp,
            tc.tile_pool(name="sq", bufs=1) as sqp,
            tc.tile_pool(name="e", bufs=1) as ep,
            tc.tile_pool(name="small", bufs=1) as small,
            tc.tile_pool(name="loss", bufs=1) as lossp,
            tc.tile_pool(name="ps_g", bufs=3, space="PSUM") as ps_g,
            tc.tile_pool(name="ps_rc", bufs=1, space="PSUM") as ps_rc,
            tc.tile_pool(name="ps_sm", bufs=1, space="PSUM") as ps_sm,
        ):
            # ---- inputs (combo first: it gates the critical chain) ----
            combo = featp.tile([128, 768], U8, name="combo", tag="combo")
            f2 = featp.tile([128, 1024], FP8, name="f2", tag="f2")
            f1 = featp.tile([128, 1024], FP8, name="f1", tag="f1")
            cnt = featp.tile([128, 512], BF16, name="cnt", tag="cnt")
            # issue from four different engines so the ~650ns dma_start
            # executions overlap instead of serializing on Sync
            nc.sync.dma_start(out=combo[:, 0:512], in_=combod[:, 0:512])
            nc.scalar.dma_start(out=f2, in_=feat2[:, :])
            nc.sync.dma_start(out=f1, in_=feat1[:, :])
            nc.gpsimd.dma_start(out=cnt, in_=cntd[:, :])
            nc.gpsimd.dma_start(out=combo[:, 512:768], in_=combod[:, 512:768])
            f2m = combo[:, 0:256].bitcast(FP8)
            f1m = combo[:, 256:512].bitcast(FP8)
            cntm_bc = combo[:, 512:768].bitcast(BF16)

            # ---- constants ----
            ones_col = const.tile([128, 1], BF16, name="ones_col", tag="oc")
            nc.vector.memset(ones_col, 1.0)
            ones_row = const.tile([1, 128], BF16, name="ones_row", tag="orow")
            nc.vector.memset(ones_row, 1.0)
            one_1x1 = const.tile([1, 1], F32, name="one11", tag="one11")
            nc.vector.memset(one_1x1, 1.0)
            eps_col = const.tile([128, 1], F32, name="eps_col", tag="eps")
            nc.vector.memset(eps_col, EPS)
            ln_ibw_col = const.tile([128, 1], F32, name="ln_ibw", tag="libw")
            nc.vector.memset(ln_ibw_col, math.log(1.0 / BW))
            ln_half_col = const.tile([128, 1], F32, name="ln_half", tag="lhalf")
            nc.vector.memset(ln_half_col, math.log(0.5))
            ones8 = const.tile([128, 2], FP8, name="ones8", tag="ones8")
            nc.vector.memset(ones8, 1.0)
            ones8r = ones8.rearrange("q (i o) -> q i o", i=2)

            # shared PSUM bank-tiles (sub-sliced; each distinct tile = 1 bank)
            smallrow = ps_sm.tile([1, 512], F32, name="smallrow", tag="smrow")
            smallcol = ps_sm.tile([128, 512], F32, name="smallcol", tag="smcol")

            # ==== critical m-slice norm path (floor 0) ====
            sqm = sqp.tile([128, 512], BF16, name="sqm", tag="sqm")
            sq2m, sq1m = sqm[:, 0:256], sqm[:, 256:512]
            nc.vector.tensor_tensor(out=sq2m, in0=f2m, in1=f2m, op=MUL)
            nc.vector.tensor_tensor(out=sq1m, in0=f1m, in1=f1m, op=MUL)
            ssm_ps = smallrow[0:1, 0:256]
            for i in range(2):
                nc.tensor.matmul(
                    out=ssm_ps[0:1, 0:128], lhsT=ones_col,
                    rhs=sq2m[:, 128 * i : 128 * (i + 1)],
                    start=(i == 0), stop=(i == 1),
                )
            for i in range(2):
                nc.tensor.matmul(
                    out=ssm_ps[0:1, 128:256], lhsT=ones_col,
                    rhs=sq1m[:, 128 * i : 128 * (i + 1)],
                    start=(i == 0), stop=(i == 1),
                )
            lnm = small.tile([1, 256], F32, name="lnm", tag="lnm")
            nc.scalar.activation(out=lnm, in_=ssm_ps, func=LN)
            invm_row = small.tile([1, 256], BF16, name="invm_row", tag="invm")
            nc.scalar.activation(out=invm_row, in_=lnm, func=EXP, scale=-0.5)
            bc_ps = smallcol[:, 0:256]
            nc.tensor.matmul(out=bc_ps, lhsT=ones_row, rhs=invm_row)
            # pre-scaled m-slices (gram rhs): f2ms = t2m * inv2m, f1ms = t1m * inv1m
            f2ms = featp.tile([128, 256], FP8, name="f2ms", tag="f2ms")
            f1ms = featp.tile([128, 256], FP8, name="f1ms", tag="f1ms")
            for i in range(2):
                isl = slice(128 * i, 128 * (i + 1))
                nc.vector.tensor_tensor(
                    out=f1ms[:, isl], in0=f1m[:, isl], in1=bc_ps[:, 128:256], op=MUL
                )
            for i in range(2):
                isl = slice(128 * i, 128 * (i + 1))
                nc.vector.tensor_tensor(
                    out=f2ms[:, isl], in0=f2m[:, isl], in1=bc_ps[:, 0:128], op=MUL
                )
            dprod = sqp.tile([128, 256], BF16, name="dprod", tag="dprod")
            nc.gpsimd.tensor_tensor(out=dprod, in0=f2ms, in1=f1ms, op=MUL)

            # ==== full-range sumsq, DIRECTLY in column form ====
            # ss_col[s' in a-block] = sum_c sq[c, s'] via 1-column DoubleRow
            # matmuls (lhsT = squares, rhs = ones): no (1,512) Lns, no PE
            # row->col transposes.  t2 squares on Vector (gates the et side),
            # t1 squares on GpSimd (slower but parallel; em side comes later).
            sq2 = sqp.tile([128, 1024], FP8, name="sq2", tag="sq2")
            sq1 = sqp.tile([128, 1024], FP8, name="sq1", tag="sq1")
            nc.vector.tensor_tensor(
                out=sq2[:, 0:512], in0=f2[:, 0:512], in1=f2[:, 0:512], op=MUL
            )
            nc.vector.tensor_tensor(
                out=sq2[:, 512:1024], in0=f2[:, 512:1024], in1=f2[:, 512:1024], op=MUL
            )
            nc.vector.tensor_tensor(
                out=sq1[:, 0:512], in0=f1[:, 0:512], in1=f1[:, 0:512], op=MUL
            )
            nc.vector.tensor_tensor(
                out=sq1[:, 512:1024], in0=f1[:, 512:1024], in1=f1[:, 512:1024], op=MUL
            )
            cntm = lossp.tile([128, 128], F32, name="cntm", tag="cntm")
            nc.gpsimd.tensor_copy(out=cntm, in_=cntm_bc)
            lncol = small.tile([128, 8], F32, name="lncol", tag="lncol")
            invbw = small.tile([128, 8], F32, name="invbw", tag="invbw")
            inv2bw, inv1bw = invbw[:, 0:4], invbw[:, 4:8]
            ss_ps = ps_sm.tile([128, 8], F32, name="ss_ps", tag="ssps")
            with tc.tile_wait_until(FLOOR_FULL):
                sq2r = sq2.rearrange("q (i s) -> q i s", i=2)
                for a in range(4):
                    nc.tensor.matmul(
                        out=ss_ps[:, a : a + 1],
                        lhsT=sq2r[:, :, 128 * a : 128 * (a + 1)],
                        rhs=ones8r,
                        perf_mode=mybir.MatmulPerfMode.DoubleRow,
                    )
                nc.scalar.activation(out=lncol[:, 0:4], in_=ss_ps[:, 0:4], func=LN)
                nc.scalar.activation(
                    out=inv2bw, in_=lncol[:, 0:4], func=EXP, scale=-0.5,
                    bias=ln_ibw_col,
                )
            with tc.tile_wait_until(FLOOR_FULL2):
                sq1r = sq1.rearrange("q (i s) -> q i s", i=2)
                for a in range(4):
                    nc.tensor.matmul(
                        out=ss_ps[:, 4 + a : 5 + a],
                        lhsT=sq1r[:, :, 128 * a : 128 * (a + 1)],
                        rhs=ones8r,
                        perf_mode=mybir.MatmulPerfMode.DoubleRow,
                    )
                nc.scalar.activation(out=lncol[:, 4:8], in_=ss_ps[:, 4:8], func=LN)
                nc.scalar.activation(
                    out=inv1bw, in_=lncol[:, 4:8], func=EXP, scale=-0.5,
                    bias=ln_ibw_col,
                )

            # ==== Grams via fp8 DoubleRow (floor 2) ====
            # Per-block row scales are folded in by Vector tensor_scalar ops
            # (PSUM -> SBUF f32), so each side needs ONE big (128,512) exp
            # instead of four small ones (Scalar was the pipeline bottleneck).
            with tc.tile_wait_until(FLOOR_GRAM):
                et = ep.tile([128, 512], BF16, name="et", tag="et")
                em = ep.tile([128, 512], BF16, name="em", tag="em")
                gsc_et = ep.tile([128, 512], F32, name="gsc_et", tag="gsc_et")
                gsc_em = ep.tile([128, 512], F32, name="gsc_em", tag="gsc_em")
                f2r = f2.rearrange("q (i s) -> q i s", i=2)
                f1r = f1.rearrange("q (i s) -> q i s", i=2)
                f2msr = f2ms.rearrange("q (i j) -> q i j", i=2)
                f1msr = f1ms.rearrange("q (i j) -> q i j", i=2)
                for dst, gsc, lhs_r, rhs_r, sc in (
                    (et, gsc_et, f2r, f1msr, inv2bw),
                    (em, gsc_em, f1r, f2msr, inv1bw),
                ):
                    for a in range(4):
                        g_ps = ps_g.tile([128, 128], F32, name="g_ps", tag="g_ps")
                        nc.tensor.matmul(
                            out=g_ps,
                            lhsT=lhs_r[:, :, 128 * a : 128 * (a + 1)],
                            rhs=rhs_r,
                            perf_mode=mybir.MatmulPerfMode.DoubleRow,
                        )
                        nc.vector.tensor_scalar_mul(
                            out=gsc[:, 128 * a : 128 * (a + 1)], in0=g_ps,
                            scalar1=sc[:, a : a + 1],
                        )
                    nc.scalar.activation(
                        out=dst[:, 0:256], in_=gsc[:, 0:256], func=EXP
                    )
                    nc.scalar.activation(
                        out=dst[:, 256:512], in_=gsc[:, 256:512], func=EXP
                    )

            # ==== diag path (floor 3; only needed by the final Ln) ====
            with tc.tile_wait_until(FLOOR_DIAG):
                dps = smallrow[0:1, 256:384]
                for i in range(2):
                    nc.tensor.matmul(
                        out=dps, lhsT=ones_col, rhs=dprod[:, 128 * i : 128 * (i + 1)],
                        start=(i == 0), stop=(i == 1),
                    )
                drow = small.tile([1, 128], F32, name="drow", tag="drow")
                nc.vector.tensor_copy(out=drow, in_=dps)
                dcol_ps = smallcol[:, 264:265]
                nc.tensor.transpose(out=dcol_ps, in_=drow, identity=one_1x1)
                half_dcol = small.tile([128, 1], F32, name="half_dcol", tag="hdc")
                nc.scalar.activation(
                    out=half_dcol, in_=dcol_ps, func=EXP, scale=1.0 / BW,
                    bias=ln_half_col,
                )

            # ==== RS/CS and loss (floor 4) ====
            with tc.tile_wait_until(FLOOR_LOSS):
                cs_ps = ps_rc.tile([128, 128], F32, name="cs_ps", tag="cs_ps")
                rs_ps = ps_rc.tile([128, 128], F32, name="rs_ps", tag="rs_ps")
                for a in range(4):
                    asl = slice(128 * a, 128 * (a + 1))
                    nc.tensor.matmul(
                        out=cs_ps, lhsT=et[:, asl], rhs=cnt[:, asl],
                        start=(a == 0), stop=(a == 3),
                    )
                for a in range(4):
                    asl = slice(128 * a, 128 * (a + 1))
                    nc.tensor.matmul(
                        out=rs_ps, lhsT=em[:, asl], rhs=cnt[:, asl],
                        start=(a == 0), stop=(a == 3),
                    )
                cinv = lossp.tile([128, 128], F32, name="cinv", tag="cinv")
                rinv = lossp.tile([128, 128], F32, name="rinv", tag="rinv")
                nc.vector.reciprocal_approx_fast(out=cinv, in_=cs_ps)
                nc.vector.reciprocal_approx_fast(out=rinv, in_=rs_ps)
                ssum = lossp.tile([128, 128], F32, name="ssum", tag="ssum")
                nc.vector.tensor_tensor(out=ssum, in0=rinv, in1=cinv, op=ADD)
                g = lossp.tile([128, 128], F32, name="g", tag="g")
                nc.scalar.activation(
                    out=g, in_=ssum, func=LN, scale=half_dcol, bias=eps_col
                )
                scr = lossp.tile([128, 128], BF16, name="scr", tag="scr")
                nc.vector.tensor_tensor(out=scr, in0=g, in1=cntm, op=MUL)
                tot_ps = smallrow[0:1, 384:512]
                nc.tensor.matmul(out=tot_ps, lhsT=ones_col, rhs=scr)
                tot = small.tile([1, 1], F32, name="tot", tag="totsb")
                nc.vector.tensor_reduce(out=tot, in_=tot_ps, axis=AX, op=ADD)
                nc.sync.dma_start(out=partial[:, :], in_=tot)

    nc.compile()
    return nc


_NC = None


def _run(t2_feat, t1_feat, idx, trace=False, trace_kwargs=None):
    global _NC
    if _NC is None:
        _NC = _build_program()

    t2 = np.asarray(t2_feat, np.float32).reshape(B, C, S)
    t1 = np.asarray(t1_feat, np.float32).reshape(B, C, S)
    idx = np.asarray(idx)

    counts = np.zeros((P, S), np.float32)
    np.add.at(counts, (np.arange(P)[:, None], idx), 1.0)
    cnt_dev = np.ascontiguousarray(
        counts.T.reshape(4, 128, 128).transpose(1, 0, 2).reshape(128, 512)
    ).astype(ml_dtypes.bfloat16)

    packed = {}
    for nm, t in (("2", t2), ("1", t1)):
        packed[nm] = [
            np.ascontiguousarray(
                t[b].reshape(2, 128, 512).transpose(1, 0, 2).reshape(128, 1024)
            ).astype(ml_dtypes.float8_e4m3)
            for b in range(B)
        ]

    in_maps = []
    for core in range(N_CORES):
        b, m = divmod(core, 4)
        p2, p1 = packed["2"][b], packed["1"][b]
        msl = [slice(512 * i + 128 * m, 512 * i + 128 * (m + 1)) for i in range(2)]
        f2m = np.concatenate([p2[:, s] for s in msl], axis=1)
        f1m = np.concatenate([p1[:, s] for s in msl], axis=1)
        cntm = np.ascontiguousarray(
            counts[:, 128 * m : 128 * (m + 1)].T
        ).astype(ml_dtypes.bfloat16)
        combo = np.concatenate(
            [
                f2m.view(np.uint8),
                f1m.view(np.uint8),
                cntm.view(np.uint8).reshape(128, 256),
            ],
            axis=1,
        )
        in_maps.append(
            {
                "combo": np.ascontiguousarray(combo),
                "feat2": p2,
                "feat1": p1,
                "cnt": cnt_dev,
            }
        )

    kwargs = {}
    if trace:
        kwargs = dict(trace=True, trace_kwargs=trace_kwargs or {})
    res = run_bass_kernel_spmd(_NC, in_maps, core_ids=list(range(N_CORES)), **kwargs)
    total = sum(float(r["partial"][0, 0]) for r in res.results)
    loss = -total / (P * B * K)
    return np.array(loss, dtype=np.float32), res


def kernel(t2_feat, t1_feat, idx):
    out, _ = _run(t2_feat, t1_feat, idx)
    return out


# revision 42
# speedup vs baseline: 1.0496x; 1.0036x over previous
"""Contrastive patch loss (InfoNCE over sampled voxel patches) on 8 TRN2 NeuronCores.

Math
----
Every sampled voxel index lives in [0, 512), so cs is a gather of the 512x512
Gram matrix G_b = t2n^T @ t1n: cs[k,l] = G_b[i_k, i_l].  With E_b = exp(G_b/bw)
and c_p[s] = multiplicity of voxel s in patch p:

    loss = -1/(P*B*K) * sum_{b,p,s} c_p[s] *
           log(0.5*diagE_b[s]*(1/CS_b[s,p] + 1/RS_b[s,p]) + eps)

where RS_b = E_b @ C^T and CS_b = E_b^T @ C^T.

Sharding: 8 cores = 2 batches x 4 s-row blocks (m).  Core (b,m) computes only
the m-th 128-row block of RS/CS.  The needed lhsT operands E[m,:]^T and
(E^T)[m,:]^T are produced DIRECTLY as col-block Grams (exp of t1^T@t2n[:,m]
resp. t2^T@t1n[:,m]); no PE transposes or PSUM->SBUF copies.  One SPMD
program: all m-dependence enters via host-prepared inputs (m-slice features,
m-slice counts); diag(E)[m-block] comes from the elementwise product of the
two pre-scaled m-slices.  Per-core (128,1) partials; host sums and scales.

DMA is descriptor-bound (~60ns per <=1KB descriptor per queue), so the small
inputs (both m-slices + m-counts) are packed into ONE byte tensor with 768B
rows and bitcast on device; features are fp8e4 (half the descriptors of bf16,
2x PE rate via DoubleRow).  Verified ~2e-4 rel err.
"""

import math

import ml_dtypes
import numpy as np

import concourse.bacc as bacc
import concourse.tile as tile
from concourse import hw_specs, mybir
from concourse.bass_utils import run_bass_kernel_spmd

# Pin every ACTIVATE to the one table set that holds ln+exp+copy so the kernel
# pays a single ACT_TABLE_LOAD.
_PIN_SET = "natural_log_exp_and_others"
_orig_get_tables = hw_specs.get_activation_tables


def _pinned_tables(arch):
    tabs = _orig_get_tables(arch)
    return {k: (v if k == _PIN_SET else set()) for k, v in tabs.items()}


bacc.get_activation_tables = _pinned_tables

B, C, S = 2, 256, 512
P, K = 128, 512
BW = 0.05
EPS = 1e-5
N_CORES = 8
F32 = mybir.dt.float32
BF16 = mybir.dt.bfloat16
FP8 = mybir.dt.float8e4
U8 = mybir.dt.uint8
AX = mybir.AxisListType.X
MUL = mybir.AluOpType.mult
ADD = mybir.AluOpType.add
EXP = mybir.ActivationFunctionType.Exp
LN = mybir.ActivationFunctionType.Ln


FLOOR_FULL = 0.008
FLOOR_FULL2 = 0.008
FLOOR_GRAM = 0.010
FLOOR_DIAG = 0.012
FLOOR_LOSS = 0.014


def _build_program():
    nc = bacc.Bacc("TRN2", target_bir_lowering=False, debug=False, num_devices=N_CORES)

    # combo rows: [f2m fp8 256B | f1m fp8 256B | cntm bf16 256B]
    combod = nc.dram_tensor("combo", [128, 768], U8, kind="ExternalInput")
    # features packed (q, i*512+s): channel c = 128*i + q
    feat2 = nc.dram_tensor("feat2", [128, 1024], FP8, kind="ExternalInput")
    feat1 = nc.dram_tensor("feat1", [128, 1024], FP8, kind="ExternalInput")
    # counts^T blocks (q, a*128+p) = counts[p, 128*a+q]
    cntd = nc.dram_tensor("cnt", [128, 512], BF16, kind="ExternalInput")
    partial = nc.dram_tensor("partial", [1, 1], F32, kind="ExternalOutput")

    with tile.TileContext(nc) as tc:
        with (
            tc.tile_pool(name="const", bufs=1) as const,
            tc.tile_pool(name="feat", bufs=1) as featp,
            tc.tile_pool(name="sq", bufs=1) as sqp,
            tc.tile_pool(name="e", bufs=1) as ep,
            tc.tile_pool(name="small", bufs=1) as small,
            tc.tile_pool(name="loss", bufs=1) as lossp,
            tc.tile_pool(name="ps_g", bufs=4, space="PSUM") as ps_g,
            tc.tile_pool(name="ps_rc", bufs=1, space="PSUM") as ps_rc,
            tc.tile_pool(name="ps_sm", bufs=1, space="PSUM") as ps_sm,
        ):
            # ---- inputs (combo first: it gates the critical chain) ----
            combo = featp.tile([128, 768], U8, name="combo", tag="combo")
            f2 = featp.tile([128, 1024], FP8, name="f2", tag="f2")
            f1 = featp.tile([128, 1024], FP8, name="f1", tag="f1")
            cnt = featp.tile([128, 512], BF16, name="cnt", tag="cnt")
            # issue from four different engines so the ~650ns dma_start
            # executions overlap instead of serializing on Sync
            nc.sync.dma_start(out=combo[:, 0:512], in_=combod[:, 0:512])
            nc.scalar.dma_start(out=f2, in_=feat2[:, :])
            nc.sync.dma_start(out=f1, in_=feat1[:, :])
            nc.gpsimd.dma_start(out=cnt, in_=cntd[:, :])
            nc.gpsimd.dma_start(out=combo[:, 512:768], in_=combod[:, 512:768])
            f2m = combo[:, 0:256].bitcast(FP8)
            f1m = combo[:, 256:512].bitcast(FP8)
            cntm_bc = combo[:, 512:768].bitcast(BF16)

            # ---- constants ----
            ones_col = const.tile([128, 1], BF16, name="ones_col", tag="oc")
            nc.vector.memset(ones_col, 1.0)
            ones_row = const.tile([1, 128], BF16, name="ones_row", tag="orow")
            nc.vector.memset(ones_row, 1.0)
            one_1x1 = const.tile([1, 1], F32, name="one11", tag="one11")
            nc.vector.memset(one_1x1, 1.0)
            eps_col = const.tile([128, 1], F32, name="eps_col", tag="eps")
            nc.vector.memset(eps_col, EPS)
            ln_ibw_col = const.tile([128, 1], F32, name="ln_ibw", tag="libw")
            nc.vector.memset(ln_ibw_col, math.log(1.0 / BW))
            ln_half_col = const.tile([128, 1], F32, name="ln_half", tag="lhalf")
            nc.vector.memset(ln_half_col, math.log(0.5))
            ones8 = const.tile([128, 2], FP8, name="ones8", tag="ones8")
            nc.vector.memset(ones8, 1.0)
            ones8r = ones8.rearrange("q (i o) -> q i o", i=2)

            # ONE shared PSUM bank, time-multiplexed (all cross-uses are
            # naturally time-ordered, so tile-granular deps add no stalls)
            shared = ps_sm.tile([128, 512], F32, name="sharedps", tag="shps")

            # ==== critical m-slice norm path (floor 0) ====
            sqm = sqp.tile([128, 512], BF16, name="sqm", tag="sqm")
            sq2m, sq1m = sqm[:, 0:256], sqm[:, 256:512]
            nc.vector.tensor_tensor(out=sq2m, in0=f2m, in1=f2m, op=MUL)
            nc.vector.tensor_tensor(out=sq1m, in0=f1m, in1=f1m, op=MUL)
            ssm_ps = shared[0:1, 256:512]
            for i in range(2):
                nc.tensor.matmul(
                    out=ssm_ps[0:1, 0:128], lhsT=ones_col,
                    rhs=sq2m[:, 128 * i : 128 * (i + 1)],
                    start=(i == 0), stop=(i == 1),
                )
            for i in range(2):
                nc.tensor.matmul(
                    out=ssm_ps[0:1, 128:256], lhsT=ones_col,
                    rhs=sq1m[:, 128 * i : 128 * (i + 1)],
                    start=(i == 0), stop=(i == 1),
                )
            lnm = small.tile([1, 256], F32, name="lnm", tag="lnm")
            nc.scalar.activation(out=lnm, in_=ssm_ps, func=LN)
            invm_row = small.tile([1, 256], BF16, name="invm_row", tag="invm")
            nc.scalar.activation(out=invm_row, in_=lnm, func=EXP, scale=-0.5)
            bc_ps = shared[:, 0:256]
            nc.tensor.matmul(out=bc_ps, lhsT=ones_row, rhs=invm_row)
            # pre-scaled m-slices (gram rhs): f2ms = t2m * inv2m, f1ms = t1m * inv1m
            f2ms = featp.tile([128, 256], FP8, name="f2ms", tag="f2ms")
            f1ms = featp.tile([128, 256], FP8, name="f1ms", tag="f1ms")
            for i in range(2):
                isl = slice(128 * i, 128 * (i + 1))
                nc.vector.tensor_tensor(
                    out=f1ms[:, isl], in0=f1m[:, isl], in1=bc_ps[:, 128:256], op=MUL
                )
            for i in range(2):
                isl = slice(128 * i, 128 * (i + 1))
                nc.vector.tensor_tensor(
                    out=f2ms[:, isl], in0=f2m[:, isl], in1=bc_ps[:, 0:128], op=MUL
                )
            dprod = sqp.tile([128, 256], BF16, name="dprod", tag="dprod")
            nc.gpsimd.tensor_tensor(out=dprod, in0=f2ms, in1=f1ms, op=MUL)

            # ==== full-range sumsq, DIRECTLY in column form ====
            # ss_col[s' in a-block] = sum_c sq[c, s'] via 1-column DoubleRow
            # matmuls (lhsT = squares, rhs = ones): no (1,512) Lns, no PE
            # row->col transposes.  t2 squares on Vector (gates the et side),
            # t1 squares on GpSimd (slower but parallel; em side comes later).
            sq2 = sqp.tile([128, 1024], FP8, name="sq2", tag="sq2")
            sq1 = sqp.tile([128, 1024], FP8, name="sq1", tag="sq1")
            nc.vector.tensor_tensor(
                out=sq2[:, 0:512], in0=f2[:, 0:512], in1=f2[:, 0:512], op=MUL
            )
            nc.vector.tensor_tensor(
                out=sq2[:, 512:1024], in0=f2[:, 512:1024], in1=f2[:, 512:1024], op=MUL
            )
            nc.vector.tensor_tensor(
                out=sq1[:, 0:512], in0=f1[:, 0:512], in1=f1[:, 0:512], op=MUL
            )
            nc.vector.tensor_tensor(
                out=sq1[:, 512:1024], in0=f1[:, 512:1024], in1=f1[:, 512:1024], op=MUL
            )
            cntm = lossp.tile([128, 128], F32, name="cntm", tag="cntm")
            nc.gpsimd.tensor_copy(out=cntm, in_=cntm_bc)
            lncol = small.tile([128, 8], F32, name="lncol", tag="lncol")
            invbw = small.tile([128, 8], F32, name="invbw", tag="invbw")
            inv2bw, inv1bw = invbw[:, 0:4], invbw[:, 4:8]
            ss_ps = ps_sm.tile([128, 8], F32, name="ss_ps", tag="ssps")
            with tc.tile_wait_until(FLOOR_FULL):
                sq2r = sq2.rearrange("q (i s) -> q i s", i=2)
                for a in range(4):
                    nc.tensor.matmul(
                        out=ss_ps[:, a : a + 1],
                        lhsT=sq2r[:, :, 128 * a : 128 * (a + 1)],
                        rhs=ones8r,
                        perf_mode=mybir.MatmulPerfMode.DoubleRow,
                    )
                nc.scalar.activation(out=lncol[:, 0:4], in_=ss_ps[:, 0:4], func=LN)
                nc.scalar.activation(
                    out=inv2bw, in_=lncol[:, 0:4], func=EXP, scale=-0.5,
                    bias=ln_ibw_col,
                )
            with tc.tile_wait_until(FLOOR_FULL2):
                sq1r = sq1.rearrange("q (i s) -> q i s", i=2)
                for a in range(4):
                    nc.tensor.matmul(
                        out=ss_ps[:, 4 + a : 5 + a],
                        lhsT=sq1r[:, :, 128 * a : 128 * (a + 1)],
                        rhs=ones8r,
                        perf_mode=mybir.MatmulPerfMode.DoubleRow,
                    )
                nc.scalar.activation(out=lncol[:, 4:8], in_=ss_ps[:, 4:8], func=LN)
                nc.scalar.activation(
                    out=inv1bw, in_=lncol[:, 4:8], func=EXP, scale=-0.5,
                    bias=ln_ibw_col,
                )

            # ==== Grams via fp8 DoubleRow (floor 2) ====
            # Per-block row scales are folded in by Vector tensor_scalar ops
            # (PSUM -> SBUF f32), so each side needs ONE big (128,512) exp
            # instead of four small ones (Scalar was the pipeline bottleneck).
            with tc.tile_wait_until(FLOOR_GRAM):
                et = ep.tile([128, 512], BF16, name="et", tag="et")
                em = ep.tile([128, 512], BF16, name="em", tag="em")
                gsc_et = ep.tile([128, 512], F32, name="gsc_et", tag="gsc_et")
                gsc_em = ep.tile([128, 512], F32, name="gsc_em", tag="gsc_em")
                f2r = f2.rearrange("q (i s) -> q i s", i=2)
                f1r = f1.rearrange("q (i s) -> q i s", i=2)
                f2msr = f2ms.rearrange("q (i j) -> q i j", i=2)
                f1msr = f1ms.rearrange("q (i j) -> q i j", i=2)
                for dst, gsc, lhs_r, rhs_r, sc in (
                    (et, gsc_et, f2r, f1msr, inv2bw),
                    (em, gsc_em, f1r, f2msr, inv1bw),
                ):
                    for a in range(4):
                        g_ps = ps_g.tile([128, 128], F32, name="g_ps", tag="g_ps")
                        nc.tensor.matmul(
                            out=g_ps,
                            lhsT=lhs_r[:, :, 128 * a : 128 * (a + 1)],
                            rhs=rhs_r,
                            perf_mode=mybir.MatmulPerfMode.DoubleRow,
                        )
                        nc.vector.tensor_scalar_mul(
                            out=gsc[:, 128 * a : 128 * (a + 1)], in0=g_ps,
                            scalar1=sc[:, a : a + 1],
                        )
                    nc.scalar.activation(
                        out=dst[:, 0:256], in_=gsc[:, 0:256], func=EXP
                    )
                    nc.scalar.activation(
                        out=dst[:, 256:512], in_=gsc[:, 256:512], func=EXP
                    )

            # ==== diag path (floor 3; only needed by the final Ln) ====
            with tc.tile_wait_until(FLOOR_DIAG):
                dps = shared[0:1, 256:384]
                for i in range(2):
                    nc.tensor.matmul(
                        out=dps, lhsT=ones_col, rhs=dprod[:, 128 * i : 128 * (i + 1)],
                        start=(i == 0), stop=(i == 1),
                    )
                drow = small.tile([1, 128], F32, name="drow", tag="drow")
                nc.vector.tensor_copy(out=drow, in_=dps)
                dcol_ps = shared[:, 508:509]
                nc.tensor.transpose(out=dcol_ps, in_=drow, identity=one_1x1)
                half_dcol = small.tile([128, 1], F32, name="half_dcol", tag="hdc")
                nc.scalar.activation(
                    out=half_dcol, in_=dcol_ps, func=EXP, scale=1.0 / BW,
                    bias=ln_half_col,
                )

            # ==== RS/CS and loss (floor 4) ====
            with tc.tile_wait_until(FLOOR_LOSS):
                cs_ps = ps_rc.tile([128, 128], F32, name="cs_ps", tag="cs_ps")
                rs_ps = ps_rc.tile([128, 128], F32, name="rs_ps", tag="rs_ps")
                for a in range(4):
                    asl = slice(128 * a, 128 * (a + 1))
                    nc.tensor.matmul(
                        out=cs_ps, lhsT=et[:, asl], rhs=cnt[:, asl],
                        start=(a == 0), stop=(a == 3),
                    )
                for a in range(4):
                    asl = slice(128 * a, 128 * (a + 1))
                    nc.tensor.matmul(
                        out=rs_ps, lhsT=em[:, asl], rhs=cnt[:, asl],
                        start=(a == 0), stop=(a == 3),
                    )
                cinv = lossp.tile([128, 128], F32, name="cinv", tag="cinv")
                rinv = lossp.tile([128, 128], F32, name="rinv", tag="rinv")
                nc.vector.reciprocal_approx_fast(out=cinv, in_=cs_ps)
                nc.vector.reciprocal_approx_fast(out=rinv, in_=rs_ps)
                ssum = lossp.tile([128, 128], F32, name="ssum", tag="ssum")
                nc.vector.tensor_tensor(out=ssum, in0=rinv, in1=cinv, op=ADD)
                g = lossp.tile([128, 128], F32, name="g", tag="g")
                nc.scalar.activation(
                    out=g, in_=ssum, func=LN, scale=half_dcol, bias=eps_col
                )
                scr = lossp.tile([128, 128], BF16, name="scr", tag="scr")
                nc.vector.tensor_tensor(out=scr, in0=g, in1=cntm, op=MUL)
                tot_ps = shared[0:1, 380:508]
                nc.tensor.matmul(out=tot_ps, lhsT=ones_col, rhs=scr)
                tot = small.tile([1, 1], F32, name="tot", tag="totsb")
                nc.vector.tensor_reduce(out=tot, in_=tot_ps, axis=AX, op=ADD)
                nc.sync.dma_start(out=partial[:, :], in_=tot)

    nc.compile()
    return nc


_NC = None


def _run(t2_feat, t1_feat, idx, trace=False, trace_kwargs=None):
    global _NC
    if _NC is None:
        _NC = _build_program()

    t2 = np.asarray(t2_feat, np.float32).reshape(B, C, S)
    t1 = np.asarray(t1_feat, np.float32).reshape(B, C, S)
    idx = np.asarray(idx)

    counts = np.zeros((P, S), np.float32)
    np.add.at(counts, (np.arange(P)[:, None], idx), 1.0)
    cnt_dev = np.ascontiguousarray(
        counts.T.reshape(4, 128, 128).transpose(1, 0, 2).reshape(128, 512)
    ).astype(ml_dtypes.bfloat16)

    packed = {}
    for nm, t in (("2", t2), ("1", t1)):
        packed[nm] = [
            np.ascontiguousarray(
                t[b].reshape(2, 128, 512).transpose(1, 0, 2).reshape(128, 1024)
            ).astype(ml_dtypes.float8_e4m3)
            for b in range(B)
        ]

    in_maps = []
    for core in range(N_CORES):
        b, m = divmod(core, 4)
        p2, p1 = packed["2"][b], packed["1"][b]
        msl = [slice(512 * i + 128 * m, 512 * i + 128 * (m + 1)) for i in range(2)]
        f2m = np.concatenate([p2[:, s] for s in msl], axis=1)
        f1m = np.concatenate([p1[:, s] for s in msl], axis=1)
        cntm = np.ascontiguousarray(
            counts[:, 128 * m : 128 * (m + 1)].T
        ).astype(ml_dtypes.bfloat16)
        combo = np.concatenate(
            [
                f2m.view(np.uint8),
                f1m.view(np.uint8),
                cntm.view(np.uint8).reshape(128, 256),
            ],
            axis=1,
        )
        in_maps.append(
            {
                "combo": np.ascontiguousarray(combo),
                "feat2": p2,
                "feat1": p1,
                "cnt": cnt_dev,
            }
        )

    kwargs = {}
    if trace:
        kwargs = dict(trace=True, trace_kwargs=trace_kwargs or {})
    res = run_bass_kernel_spmd(_NC, in_maps, core_ids=list(range(N_CORES)), **kwargs)
    total = sum(float(r["partial"][0, 0]) for r in res.results)
    loss = -total / (P * B * K)
    return np.array(loss, dtype=np.float32), res


def kernel(t2_feat, t1_feat, idx):
    out, _ = _run(t2_feat, t1_feat, idx)
    return out
